# revision 22
# baseline (speedup 1.0000x reference)
"""Trainium2 Bass kernel for nn_Model12 (gnn_message_passing).

Strategy
--------
Host (numpy, tiny): the 20-node graph stage (2 TransformerConv + norms +
global-attention gate -> V) collapses, for the per-move stage, into per-node
lookup tables:
    attack  slot j: o = TAs[a_src] + TAd[a_dst] + a_armies*va   (TAs has atk_b folded)
    transfer slot j: o = TTs[t_src] + TTd[t_dst] + t_armies*vt  (TTs has trf_b folded)
    deploy  slot j: o = TD [d_tgt]               + d_armies*vd  (TD  has dep_b folded)
The per-move GraphNorm + accumulator linear + sum reduce to per-move stats
    S[m,c] = sum_j o_j[c],  Q[m,c] = sum_j o_j[c]^2
    mean = S/24, var = Q/24 - (2a-a^2)*mean^2
    p[m] = sum_c gp[c]*S[m,c]/sqrt(var[m,c]+eps) + Cp
with gp = acc_W*n4_w*(1-n4_a), Cp = 24*(acc_W.n4_b + acc_b).

Device (8 NeuronCores, move-axis data parallel, 6250 moves/core):
Gathers are done as bf16 matmuls against host-uploaded one-hot row blocks
(uint8 in HBM, cast to bf16 by SWDGE DMA).  7 "gather" matmuls/chunk produce
all 24 per-slot o-vectors in PSUM, squares go through DVE/ACT, 4 "reduce"
matmuls accumulate Q, 1 counts-matmul produces S(=mean), then a 128-partition
normalization block computes p for 4 chunks at a time.  Final log_softmax on
host over the gathered 50000 logits.
"""
import numpy as np
import ml_dtypes

N = 20
EPS = 1e-5
N_CORES = 8
M_TOTAL = 50000
MC = M_TOTAL // N_CORES          # 6250 moves per core
CHUNK = 512
NCHUNK = (MC + CHUNK - 1) // CHUNK   # 13
MC_PAD = NCHUNK * CHUNK              # 6656
ARM_SCALE = 25.5                     # armies u8 = round(armies * 25.5), armies in [0,10)
SUM_SCALE = 255.0 / 80.0             # sum-armies u8 scale

# pass slot composition: ('a'|'t'|'d', slot index)
PASS_SPECS = [
    [('a', 0), ('a', 1), ('a', 2)],
    [('a', 3), ('a', 4), ('a', 5)],
    [('a', 6), ('a', 7), ('t', 0)],
    [('t', 1), ('t', 2), ('t', 3)],
    [('t', 4), ('t', 5), ('t', 6)],
    [('t', 7), ('d', 0), ('d', 1)],
    [('d', 2), ('d', 3), ('d', 4), ('d', 5), ('d', 6), ('d', 7)],
]
# rows per pass: one-hot blocks (20/slot for 'd', 40/slot for 'a'/'t') then 1 armies row/slot
def _pass_k(spec):
    oh = sum(40 if k in 'at' else 20 for k, _ in spec)
    return oh, oh + len(spec)
PASS_K = [_pass_k(s) for s in PASS_SPECS]           # (K_onehot, K_total)
PASS_MOUT = [20 * len(s) for s in PASS_SPECS]       # 60,60,60,60,60,80,100

# G-psum pairing: (pass a at rows 0.., pass b at rows 64..) -> sq tile -> reduce matmul
#   R0 = sq(G0|G1), R1 = sq(G2|G3), R2 = sq(G4 | G5 rows0:60), R3 = sq(G5 rows60:80 | G6)
GROUPS = [list(range(g, min(g + 4, NCHUNK))) for g in range(0, NCHUNK, 4)]

_BF16 = ml_dtypes.bfloat16


# ----------------------------------------------------------------- host math
def _linear(x, W, b):
    return x @ W.T + b


def _graph_norm(x, w, b, a, axis=0):
    mean = x.mean(axis=axis, keepdims=True)
    out = x - a * mean
    var = (out * out).mean(axis=axis, keepdims=True)
    return out / np.sqrt(var + EPS) * w + b


def _tconv(x, src, dst, Wq, bq, Wk, bk, Wv, bv, Ws, bs, Wbeta):
    d = Wq.shape[0]
    q, k, v = _linear(x, Wq, bq), _linear(x, Wk, bk), _linear(x, Wv, bv)
    score = (q[dst] * k[src]).sum(-1) / np.sqrt(np.float32(d))
    m = np.full(N, -np.inf, dtype=x.dtype)
    np.maximum.at(m, dst, score)
    e = np.exp(score - m[dst])
    z = np.zeros(N, dtype=x.dtype)
    np.add.at(z, dst, e)
    alpha = e / z[dst]
    out = np.zeros((N, v.shape[1]), dtype=x.dtype)
    np.add.at(out, dst, alpha[:, None] * v[src])
    skip = _linear(x, Ws, bs)
    beta = 1.0 / (1.0 + np.exp(-(np.concatenate([out, skip, out - skip], -1) @ Wbeta.T)))
    return beta * skip + (1.0 - beta) * out


def _graph_stage(f):
    x1 = f['x1']
    src, dst = f['edges'][0], f['edges'][1]
    relu = lambda v: np.maximum(v, 0.0)
    xa = relu(_tconv(x1, src, dst, f['g1_Wq'], f['g1_bq'], f['g1_Wk'], f['g1_bk'],
                     f['g1_Wv'], f['g1_bv'], f['g1_Ws'], f['g1_bs'], f['g1_Wbeta']))
    xa = _graph_norm(xa, f['n1_w'], f['n1_b'], f['n1_a'])
    xb = relu(_tconv(np.concatenate([x1, xa], 1), src, dst,
                     f['g2_Wq'], f['g2_bq'], f['g2_Wk'], f['g2_bk'],
                     f['g2_Wv'], f['g2_bv'], f['g2_Ws'], f['g2_bs'], f['g2_Wbeta']))
    xb = _graph_norm(xb, f['n2_w'], f['n2_b'], f['n2_a'])
    xc = np.concatenate([x1, xa, xb], 1)
    gate = _linear(xc, f['gate_W'], f['gate_b'])
    gate = gate - gate.max(0, keepdims=True)
    alpha = np.exp(gate) / np.exp(gate).sum(0, keepdims=True)
    xg = (alpha * _linear(xc, f['nn_W'], f['nn_b'])).sum(0)
    xg = _graph_norm(xg, f['n3_w'], f['n3_b'], f['n3_a'])
    V = np.tanh(_linear(np.concatenate([relu(xg), f['x2']]), f['lin1_W'], f['lin1_b']))[0]

    X = np.concatenate([xa, xb], 1)
    col0 = x1[:, 0]
    mask = (x1[:, 5:15] == 1.0).astype(x1.dtype)
    s = (mask * col0[:, None]).sum(0)
    cnt = mask.sum(0)
    tmp = np.where(mask > 0,
                   (s[None, :] - col0[:, None]) / np.where(cnt > 1, cnt - 1.0, 1.0),
                   0.0)

    nodef = np.concatenate([x1, tmp, X], 1)          # (N, 65) order x1,tmp,X

    def split_pair(W):
        Ws_ = np.concatenate([W[:, 0:15], W[:, 30:40], W[:, 50:90]], 1)
        Wd_ = np.concatenate([W[:, 15:30], W[:, 40:50], W[:, 90:130]], 1)
        return Ws_, Wd_

    aWs, aWd = split_pair(f['atk_W'])
    tWs, tWd = split_pair(f['trf_W'])
    T = {}
    T['TAs'] = nodef @ aWs.T + f['atk_b']            # atk_b folded into src table
    T['TAd'] = nodef @ aWd.T - 0.7 * f['atk_W'][:, 131][None, :] * (x1[:, 3] + x1[:, 4])[:, None]
    T['TTs'] = nodef @ tWs.T + f['trf_b']
    T['TTd'] = nodef @ tWd.T
    T['TD'] = nodef @ f['dep_W'][:, 0:65].T + f['dep_b']
    T['va'] = f['atk_W'][:, 130] + 0.6 * f['atk_W'][:, 131]
    T['vt'] = f['trf_W'][:, 130]
    T['vd'] = f['dep_W'][:, 65]
    acc_W, acc_b = f['acc_W'], f['acc_b']
    n4_w, n4_b, n4_a = f['n4_w'], f['n4_b'], f['n4_a']
    T['gp'] = acc_W[0] * n4_w * (1.0 - n4_a)
    T['Cp'] = np.float32(24.0 * ((acc_W[0] * n4_b).sum() + acc_b[0]))
    T['coefA'] = 2.0 * n4_a - n4_a * n4_a
    return np.float32(V), T


def _slot_tables(T, kind):
    if kind == 'a':
        return T['TAs'], T['TAd'], T['va']
    if kind == 't':
        return T['TTs'], T['TTd'], T['vt']
    return T['TD'], None, T['vd']


# ------------------------------------------------------------- device inputs
def _build_weights(T):
    """lhsT tensors (bf16/f32) shared by all cores."""
    w = {}
    for pi, spec in enumerate(PASS_SPECS):
        koh, k = PASS_K[pi]
        lhs = np.zeros((k, PASS_MOUT[pi]), np.float32)
        r = 0
        for si, (kind, _) in enumerate(spec):
            Ts, Td, v = _slot_tables(T, kind)
            lhs[r:r + 20, 20 * si:20 * si + 20] = Ts
            r += 20
            if Td is not None:
                lhs[r:r + 20, 20 * si:20 * si + 20] = Td
                r += 20
        for si, (kind, _) in enumerate(spec):
            _, _, v = _slot_tables(T, kind)
            lhs[koh + si, 20 * si:20 * si + 20] = v / ARM_SCALE
        w[f'w_pass{pi}'] = lhs.astype(_BF16)

    # S (counts) matmul: rows [cnt_as 20][cnt_ad 20][cnt_ts 20][cnt_td 20][cnt_d 20]
    #                         [sum_a][sum_t][sum_d]; output scaled by 1/24 -> mean
    ws = np.zeros((103, 32), np.float32)
    ws[0:20, 0:20] = T['TAs']
    ws[20:40, 0:20] = T['TAd']
    ws[40:60, 0:20] = T['TTs']
    ws[60:80, 0:20] = T['TTd']
    ws[80:100, 0:20] = T['TD']
    ws[100, 0:20] = T['va'] / SUM_SCALE
    ws[101, 0:20] = T['vt'] / SUM_SCALE
    ws[102, 0:20] = T['vd'] / SUM_SCALE
    w['w_s'] = (ws / 24.0).astype(_BF16)

    # reduce matmuls (block-sum of squares, scaled 1/24). Two row layouts:
    # wr_a for sq tiles with blocks at rows [0:60] and [64:124]
    wra = np.zeros((124, 32), np.float32)
    for r in range(60):
        wra[r, r % 20] = 1.0 / 24.0
    for r in range(64, 124):
        wra[r, (r - 64) % 20] = 1.0 / 24.0
    w['w_r_a'] = wra.astype(_BF16)
    # wr_b for sq tile with blocks at rows [0:120]
    wrb = np.zeros((120, 32), np.float32)
    for r in range(120):
        wrb[r, r % 20] = 1.0 / 24.0
    w['w_r_b'] = wrb.astype(_BF16)

    # p matmul: pv(128, n) -> p(n_chunks_in_group, n);  rows 32g+c map 24*gp[c] to col g
    wgp = np.zeros((128, 4), np.float32)
    for g in range(4):
        wgp[32 * g:32 * g + 20, g] = 24.0 * T['gp']
    w['w_gp'] = wgp.astype(_BF16)

    c2 = np.zeros((128, 1), np.float32)
    for g in range(4):
        c2[32 * g:32 * g + 20, 0] = T['coefA']
    w['c2col'] = c2
    return w


def _pack_core(idx_arrs, arm_arrs):
    """Per-core uint8 pass tensors + counts tensor.

    idx_arrs: dict kind -> (src, dst) or (tgt,) int arrays of shape (MC, 8)
    arm_arrs: dict kind -> (MC, 8) float arrays
    returns dict name -> np.ndarray
    """
    out = {}
    iota = np.arange(N, dtype=np.int32)
    armq = {k: np.clip(np.rint(arm_arrs[k] * ARM_SCALE), 0, 255).astype(np.uint8)
            for k in 'atd'}

    def pad(a):  # (MC, ...) -> (MC_PAD, ...)
        if a.shape[0] == MC_PAD:
            return a
        pw = [(0, MC_PAD - a.shape[0])] + [(0, 0)] * (a.ndim - 1)
        return np.pad(a, pw)

    for pi, spec in enumerate(PASS_SPECS):
        koh, k = PASS_K[pi]
        rows = np.zeros((k, MC_PAD), np.uint8)
        r = 0
        for kind, j in spec:
            srcs = idx_arrs[kind]
            oh = (pad(srcs[0][:, j])[None, :] == iota[:, None]).astype(np.uint8)
            rows[r:r + 20] = oh
            r += 20
            if len(srcs) > 1:
                oh = (pad(srcs[1][:, j])[None, :] == iota[:, None]).astype(np.uint8)
                rows[r:r + 20] = oh
                r += 20
        for si, (kind, j) in enumerate(spec):
            rows[koh + si] = pad(armq[kind][:, j])
        out[f'pass{pi}'] = np.ascontiguousarray(
            rows.reshape(k, NCHUNK, CHUNK).transpose(1, 0, 2))

    cnt = np.zeros((103, MC_PAD), np.uint8)
    pairs = [('a', 0), ('a', 1), ('t', 0), ('t', 1), ('d', 0)]
    for bi, (kind, which) in enumerate(pairs):
        idx = pad(idx_arrs[kind][which])              # (MC_PAD, 8)
        oh = (idx[:, :, None] == iota[None, None, :]).sum(1)  # (MC_PAD, 20) counts
        cnt[20 * bi:20 * bi + 20] = oh.T.astype(np.uint8)
    for bi, kind in enumerate('atd'):
        s = pad(arm_arrs[kind].sum(1))
        cnt[100 + bi] = np.clip(np.rint(s * SUM_SCALE), 0, 255).astype(np.uint8)
    out['counts'] = np.ascontiguousarray(
        cnt.reshape(103, NCHUNK, CHUNK).transpose(1, 0, 2))
    return out


# ------------------------------------------------------------- bass program
def _build_program():
    import concourse.bass as bass
    import concourse.tile as tile
    from concourse import bacc, mybir
    from contextlib import ExitStack

    dt = mybir.dt
    Alu = mybir.AluOpType
    Act = mybir.ActivationFunctionType

    nc = bacc.Bacc('TRN2')
    # inputs
    d_pass = []
    for pi in range(7):
        _, k = PASS_K[pi]
        d_pass.append(nc.declare_dram_parameter(
            f'pass{pi}', [NCHUNK, k, CHUNK], dt.uint8, isOutput=False))
    d_cnt = nc.declare_dram_parameter('counts', [NCHUNK, 103, CHUNK], dt.uint8, isOutput=False)
    d_w = []
    for pi in range(7):
        _, k = PASS_K[pi]
        d_w.append(nc.declare_dram_parameter(
            f'w_pass{pi}', [k, PASS_MOUT[pi]], dt.bfloat16, isOutput=False))
    d_ws = nc.declare_dram_parameter('w_s', [103, 32], dt.bfloat16, isOutput=False)
    d_wra = nc.declare_dram_parameter('w_r_a', [124, 32], dt.bfloat16, isOutput=False)
    d_wrb = nc.declare_dram_parameter('w_r_b', [120, 32], dt.bfloat16, isOutput=False)
    d_wgp = nc.declare_dram_parameter('w_gp', [128, 4], dt.bfloat16, isOutput=False)
    d_c2 = nc.declare_dram_parameter('c2col', [128, 1], dt.float32, isOutput=False)
    d_cpeps = nc.declare_dram_parameter('cp_eps', [128, 3], dt.float32, isOutput=False)
    d_out = nc.declare_dram_parameter('p2d', [NCHUNK, CHUNK], dt.float32, isOutput=True)

    with tile.TileContext(nc) as tc, ExitStack() as ctx:
        wpool = ctx.enter_context(tc.tile_pool(name='weights', bufs=1))
        rhs_pool = ctx.enter_context(tc.tile_pool(name='rhs', bufs=2))
        sq_pool = ctx.enter_context(tc.tile_pool(name='sq', bufs=2))
        pst_pool = ctx.enter_context(tc.tile_pool(name='pstage', bufs=2))
        out_pool = ctx.enter_context(tc.tile_pool(name='outp', bufs=2))
        gpsum = ctx.enter_context(tc.tile_pool(name='gpsum', bufs=1, space='PSUM'))
        spsum = ctx.enter_context(tc.tile_pool(name='spsum', bufs=1, space='PSUM'))
        qpsum = ctx.enter_context(tc.tile_pool(name='qpsum', bufs=1, space='PSUM'))
        ppsum = ctx.enter_context(tc.tile_pool(name='ppsum', bufs=1, space='PSUM'))

        # load weights/constants once
        w_sb = []
        for pi in range(7):
            _, k = PASS_K[pi]
            t = wpool.tile([k, PASS_MOUT[pi]], dt.bfloat16, tag=f'w{pi}')
            nc.sync.dma_start(t[:], d_w[pi][:, :])
            w_sb.append(t)
        ws_sb = wpool.tile([103, 32], dt.bfloat16, tag='ws')
        nc.sync.dma_start(ws_sb[:], d_ws[:, :])
        wra_sb = wpool.tile([124, 32], dt.bfloat16, tag='wra')
        nc.sync.dma_start(wra_sb[:], d_wra[:, :])
        wrb_sb = wpool.tile([120, 32], dt.bfloat16, tag='wrb')
        nc.sync.dma_start(wrb_sb[:], d_wrb[:, :])
        wgp_sb = wpool.tile([128, 4], dt.bfloat16, tag='wgp')
        nc.sync.dma_start(wgp_sb[:], d_wgp[:, :])
        c2_sb = wpool.tile([128, 1], dt.float32, tag='c2')
        nc.sync.dma_start(c2_sb[:], d_c2[:, :])
        cpeps_sb = wpool.tile([128, 3], dt.float32, tag='cpeps')
        nc.sync.dma_start(cpeps_sb[:], d_cpeps[:, :])

        for group in GROUPS:
            glen = len(group)
            s_grp = pst_pool.tile([32 * glen, CHUNK], dt.float32, tag='sgrp')
            q_grp = pst_pool.tile([32 * glen, CHUNK], dt.float32, tag='qgrp')
            for gi, c in enumerate(group):
                # --- load chunk rhs (uint8 -> bf16 cast DMA on SWDGE)
                rhs = []
                for pi in range(7):
                    _, k = PASS_K[pi]
                    t = rhs_pool.tile([k, CHUNK], dt.bfloat16, tag=f'rhs{pi}')
                    nc.gpsimd.dma_start(out=t[:], in_=d_pass[pi][c, :, :])
                    rhs.append(t)
                cnt_t = rhs_pool.tile([103, CHUNK], dt.bfloat16, tag='rhscnt')
                nc.gpsimd.dma_start(out=cnt_t[:], in_=d_cnt[c, :, :])

                # --- gather matmuls into paired PSUM tiles
                g01 = gpsum.tile([124, CHUNK], dt.float32, tag='g01')
                nc.tensor.matmul(g01[0:60, :], w_sb[0][:], rhs[0][:], start=True, stop=True)
                nc.tensor.matmul(g01[64:124, :], w_sb[1][:], rhs[1][:], start=True, stop=True,
                                 tile_position=(0, 64))
                g23 = gpsum.tile([124, CHUNK], dt.float32, tag='g23')
                nc.tensor.matmul(g23[0:60, :], w_sb[2][:], rhs[2][:], start=True, stop=True)
                nc.tensor.matmul(g23[64:124, :], w_sb[3][:], rhs[3][:], start=True, stop=True,
                                 tile_position=(0, 64))
                g45 = gpsum.tile([124, CHUNK], dt.float32, tag='g45')
                nc.tensor.matmul(g45[0:60, :], w_sb[4][:], rhs[4][:], start=True, stop=True)
                nc.tensor.matmul(g45[64:124, :], w_sb[5][:], rhs[5][:], start=True, stop=True,
                                 tile_position=(0, 64))
                g6 = gpsum.tile([120, CHUNK], dt.float32, tag='g6')
                nc.tensor.matmul(g6[:, :], w_sb[6][:], rhs[6][:], start=True, stop=True)

                # --- S (mean) matmul (per-chunk psum, copied into SBUF group buf)
                s_ps = spsum.tile([32, CHUNK], dt.float32, tag='sps')
                nc.tensor.matmul(s_ps[:], ws_sb[:], cnt_t[:], start=True, stop=True)
                nc.scalar.activation(s_grp[32 * gi:32 * gi + 32, :], s_ps[:],
                                     Act.Identity, bias=cpeps_sb[0:32, 2:3], scale=1.0)

                # --- squares (PSUM -> SBUF bf16), alternating engines
                sq0 = sq_pool.tile([124, CHUNK], dt.bfloat16, tag='sq0')
                nc.scalar.square(sq0[:], g01[:])
                sq1 = sq_pool.tile([124, CHUNK], dt.bfloat16, tag='sq1')
                nc.scalar.square(sq1[:], g23[:])
                sq2 = sq_pool.tile([124, CHUNK], dt.bfloat16, tag='sq2')
                nc.scalar.square(sq2[:], g45[:])
                sq3 = sq_pool.tile([120, CHUNK], dt.bfloat16, tag='sq3')
                nc.scalar.square(sq3[:], g6[:, :])

                # --- Q reduce matmuls (accumulate in per-chunk psum, copy to group buf)
                q_ps = qpsum.tile([32, CHUNK], dt.float32, tag='qps')
                nc.tensor.matmul(q_ps[:], wra_sb[:], sq0[:], start=True, stop=False)
                nc.tensor.matmul(q_ps[:], wra_sb[:], sq1[:], start=False, stop=False)
                nc.tensor.matmul(q_ps[:], wra_sb[:], sq2[:], start=False, stop=False)
                nc.tensor.matmul(q_ps[:], wrb_sb[:], sq3[:], start=False, stop=True)
                nc.vector.tensor_copy(q_grp[32 * gi:32 * gi + 32, :], q_ps[:])

            # --- normalization + p for the whole group (128 partitions)
            rows = 32 * glen
            ss = pst_pool.tile([rows, CHUNK], dt.float32, tag='ss')
            nc.scalar.square(ss[:], s_grp[:])                      # mean^2
            sc = pst_pool.tile([rows, CHUNK], dt.float32, tag='sc')
            nc.vector.tensor_scalar(sc[:], ss[:], c2_sb[0:rows, :], None, Alu.mult)
            vs = pst_pool.tile([rows, CHUNK], dt.float32, tag='vs')
            nc.vector.tensor_tensor(vs[:], q_grp[:], sc[:], Alu.subtract)   # var
            vc = pst_pool.tile([rows, CHUNK], dt.float32, tag='vc')
            nc.vector.tensor_scalar(vc[:], vs[:], 0.0, None, Alu.max)       # clamp >= 0
            sd = pst_pool.tile([rows, CHUNK], dt.float32, tag='sd')
            nc.scalar.activation(sd[:], vc[:], Act.Sqrt, bias=cpeps_sb[0:rows, 1:2], scale=1.0)
            rq = pst_pool.tile([rows, CHUNK], dt.float32, tag='rq')
            nc.vector.reciprocal(rq[:], sd[:])
            pv = pst_pool.tile([rows, CHUNK], dt.bfloat16, tag='pv')
            nc.vector.tensor_tensor(pv[:], s_grp[:], rq[:], Alu.mult)
            p_ps = ppsum.tile([glen, CHUNK], dt.float32, tag='pps')
            nc.tensor.matmul(p_ps[:], wgp_sb[0:rows, 0:glen], pv[:], start=True, stop=True)
            p_sb = out_pool.tile([glen, CHUNK], dt.float32, tag='psb')
            nc.scalar.activation(p_sb[:], p_ps[:], Act.Identity,
                                 bias=cpeps_sb[0:glen, 0:1], scale=1.0)
            nc.sync.dma_start(d_out[group[0]:group[0] + glen, :], p_sb[:])

    nc.finalize()
    return nc


# ------------------------------------------------------------------- runner
_CACHE = {}


def _get_program():
    if 'nc' not in _CACHE:
        _CACHE['nc'] = _build_program()
    return _CACHE['nc']


def kernel(**inputs):
    V, logp, _ = _run(inputs)
    return V, logp


def _run(inputs, trace=False, trace_kwargs=None):
    from concourse.bass_utils import run_bass_kernel_spmd

    f = {k: np.asarray(v, np.float32) if np.asarray(v).dtype != np.int32
         else np.asarray(v) for k, v in inputs.items()}
    V, T = _graph_stage(f)
    w = _build_weights(T)
    w['cp_eps'] = np.stack([np.full(128, T['Cp'], np.float32), np.full(128, EPS, np.float32), np.zeros(128, np.float32)], 1)

    in_maps = []
    for core in range(N_CORES):
        sl = slice(core * MC, (core + 1) * MC)
        idx_arrs = {'a': (f['a_src'][sl], f['a_dst'][sl]),
                    't': (f['t_src'][sl], f['t_dst'][sl]),
                    'd': (f['d_tgt'][sl],)}
        arm_arrs = {'a': f['a_armies'][sl], 't': f['t_armies'][sl],
                    'd': f['d_armies'][sl]}
        m = _pack_core(idx_arrs, arm_arrs)
        m.update(w)
        in_maps.append(m)

    nc = _get_program()
    res = run_bass_kernel_spmd(nc, in_maps, list(range(N_CORES)),
                               trace=trace, **(trace_kwargs or {}))
    p = np.concatenate([r['p2d'].reshape(-1)[:MC] for r in res.results])

    mx = p.max()
    logp = (p - mx - np.log(np.exp(p - mx).sum())).astype(np.float32)
    return np.float32(V), logp, res


# revision 23
# speedup vs baseline: 1.0066x; 1.0066x over previous
"""Trainium2 Bass kernel for nn_Model12 (gnn_message_passing).

Strategy
--------
Host (numpy, tiny): the 20-node graph stage (2 TransformerConv + norms +
global-attention gate -> V) collapses, for the per-move stage, into per-node
lookup tables:
    attack  slot j: o = TAs[a_src] + TAd[a_dst] + a_armies*va   (TAs has atk_b folded)
    transfer slot j: o = TTs[t_src] + TTd[t_dst] + t_armies*vt  (TTs has trf_b folded)
    deploy  slot j: o = TD [d_tgt]               + d_armies*vd  (TD  has dep_b folded)
The per-move GraphNorm + accumulator linear + sum reduce to per-move stats
    S[m,c] = sum_j o_j[c],  Q[m,c] = sum_j o_j[c]^2
    mean = S/24, var = Q/24 - (2a-a^2)*mean^2
    p[m] = sum_c gp[c]*S[m,c]/sqrt(var[m,c]+eps) + Cp
with gp = acc_W*n4_w*(1-n4_a), Cp = 24*(acc_W.n4_b + acc_b).

Device (8 NeuronCores, move-axis data parallel, 6250 moves/core):
Gathers are done as bf16 matmuls against host-uploaded one-hot row blocks
(uint8 in HBM, cast to bf16 by SWDGE DMA).  7 "gather" matmuls/chunk produce
all 24 per-slot o-vectors in PSUM, squares go through DVE/ACT, 4 "reduce"
matmuls accumulate Q, 1 counts-matmul produces S(=mean), then a 128-partition
normalization block computes p for 4 chunks at a time.  Final log_softmax on
host over the gathered 50000 logits.
"""
import numpy as np
import ml_dtypes

N = 20
EPS = 1e-5
N_CORES = 8
M_TOTAL = 50000
MC = M_TOTAL // N_CORES          # 6250 moves per core
CHUNK = 512
NCHUNK = (MC + CHUNK - 1) // CHUNK   # 13
MC_PAD = NCHUNK * CHUNK              # 6656
ARM_SCALE = 25.5                     # armies u8 = round(armies * 25.5), armies in [0,10)
SUM_SCALE = 255.0 / 80.0             # sum-armies u8 scale

# pass slot composition: ('a'|'t'|'d', slot index)
PASS_SPECS = [
    [('a', 0), ('a', 1), ('a', 2)],
    [('a', 3), ('a', 4), ('a', 5)],
    [('a', 6), ('a', 7), ('t', 0)],
    [('t', 1), ('t', 2), ('t', 3)],
    [('t', 4), ('t', 5), ('t', 6)],
    [('t', 7), ('d', 0), ('d', 1)],
    [('d', 2), ('d', 3), ('d', 4), ('d', 5), ('d', 6), ('d', 7)],
]
# rows per pass: one-hot blocks (20/slot for 'd', 40/slot for 'a'/'t') then 1 armies row/slot
def _pass_k(spec):
    oh = sum(40 if k in 'at' else 20 for k, _ in spec)
    return oh, oh + len(spec)
PASS_K = [_pass_k(s) for s in PASS_SPECS]           # (K_onehot, K_total)
PASS_MOUT = [20 * len(s) for s in PASS_SPECS]       # 60,60,60,60,60,80,100

# G-psum pairing: (pass a at rows 0.., pass b at rows 64..) -> sq tile -> reduce matmul
#   R0 = sq(G0|G1), R1 = sq(G2|G3), R2 = sq(G4 | G5 rows0:60), R3 = sq(G5 rows60:80 | G6)
GROUPS = [list(range(g, min(g + 4, NCHUNK))) for g in range(0, NCHUNK, 4)]

_BF16 = ml_dtypes.bfloat16


# ----------------------------------------------------------------- host math
def _linear(x, W, b):
    return x @ W.T + b


def _graph_norm(x, w, b, a, axis=0):
    mean = x.mean(axis=axis, keepdims=True)
    out = x - a * mean
    var = (out * out).mean(axis=axis, keepdims=True)
    return out / np.sqrt(var + EPS) * w + b


def _tconv(x, src, dst, Wq, bq, Wk, bk, Wv, bv, Ws, bs, Wbeta):
    d = Wq.shape[0]
    q, k, v = _linear(x, Wq, bq), _linear(x, Wk, bk), _linear(x, Wv, bv)
    score = (q[dst] * k[src]).sum(-1) / np.sqrt(np.float32(d))
    m = np.full(N, -np.inf, dtype=x.dtype)
    np.maximum.at(m, dst, score)
    e = np.exp(score - m[dst])
    z = np.zeros(N, dtype=x.dtype)
    np.add.at(z, dst, e)
    alpha = e / z[dst]
    out = np.zeros((N, v.shape[1]), dtype=x.dtype)
    np.add.at(out, dst, alpha[:, None] * v[src])
    skip = _linear(x, Ws, bs)
    beta = 1.0 / (1.0 + np.exp(-(np.concatenate([out, skip, out - skip], -1) @ Wbeta.T)))
    return beta * skip + (1.0 - beta) * out


def _graph_stage(f):
    x1 = f['x1']
    src, dst = f['edges'][0], f['edges'][1]
    relu = lambda v: np.maximum(v, 0.0)
    xa = relu(_tconv(x1, src, dst, f['g1_Wq'], f['g1_bq'], f['g1_Wk'], f['g1_bk'],
                     f['g1_Wv'], f['g1_bv'], f['g1_Ws'], f['g1_bs'], f['g1_Wbeta']))
    xa = _graph_norm(xa, f['n1_w'], f['n1_b'], f['n1_a'])
    xb = relu(_tconv(np.concatenate([x1, xa], 1), src, dst,
                     f['g2_Wq'], f['g2_bq'], f['g2_Wk'], f['g2_bk'],
                     f['g2_Wv'], f['g2_bv'], f['g2_Ws'], f['g2_bs'], f['g2_Wbeta']))
    xb = _graph_norm(xb, f['n2_w'], f['n2_b'], f['n2_a'])
    xc = np.concatenate([x1, xa, xb], 1)
    gate = _linear(xc, f['gate_W'], f['gate_b'])
    gate = gate - gate.max(0, keepdims=True)
    alpha = np.exp(gate) / np.exp(gate).sum(0, keepdims=True)
    xg = (alpha * _linear(xc, f['nn_W'], f['nn_b'])).sum(0)
    xg = _graph_norm(xg, f['n3_w'], f['n3_b'], f['n3_a'])
    V = np.tanh(_linear(np.concatenate([relu(xg), f['x2']]), f['lin1_W'], f['lin1_b']))[0]

    X = np.concatenate([xa, xb], 1)
    col0 = x1[:, 0]
    mask = (x1[:, 5:15] == 1.0).astype(x1.dtype)
    s = (mask * col0[:, None]).sum(0)
    cnt = mask.sum(0)
    tmp = np.where(mask > 0,
                   (s[None, :] - col0[:, None]) / np.where(cnt > 1, cnt - 1.0, 1.0),
                   0.0)

    nodef = np.concatenate([x1, tmp, X], 1)          # (N, 65) order x1,tmp,X

    def split_pair(W):
        Ws_ = np.concatenate([W[:, 0:15], W[:, 30:40], W[:, 50:90]], 1)
        Wd_ = np.concatenate([W[:, 15:30], W[:, 40:50], W[:, 90:130]], 1)
        return Ws_, Wd_

    aWs, aWd = split_pair(f['atk_W'])
    tWs, tWd = split_pair(f['trf_W'])
    T = {}
    T['TAs'] = nodef @ aWs.T + f['atk_b']            # atk_b folded into src table
    T['TAd'] = nodef @ aWd.T - 0.7 * f['atk_W'][:, 131][None, :] * (x1[:, 3] + x1[:, 4])[:, None]
    T['TTs'] = nodef @ tWs.T + f['trf_b']
    T['TTd'] = nodef @ tWd.T
    T['TD'] = nodef @ f['dep_W'][:, 0:65].T + f['dep_b']
    T['va'] = f['atk_W'][:, 130] + 0.6 * f['atk_W'][:, 131]
    T['vt'] = f['trf_W'][:, 130]
    T['vd'] = f['dep_W'][:, 65]
    acc_W, acc_b = f['acc_W'], f['acc_b']
    n4_w, n4_b, n4_a = f['n4_w'], f['n4_b'], f['n4_a']
    T['gp'] = acc_W[0] * n4_w * (1.0 - n4_a)
    T['Cp'] = np.float32(24.0 * ((acc_W[0] * n4_b).sum() + acc_b[0]))
    T['coefA'] = 2.0 * n4_a - n4_a * n4_a
    return np.float32(V), T


def _slot_tables(T, kind):
    if kind == 'a':
        return T['TAs'], T['TAd'], T['va']
    if kind == 't':
        return T['TTs'], T['TTd'], T['vt']
    return T['TD'], None, T['vd']


# ------------------------------------------------------------- device inputs
def _build_weights(T):
    """lhsT tensors (bf16/f32) shared by all cores."""
    w = {}
    for pi, spec in enumerate(PASS_SPECS):
        koh, k = PASS_K[pi]
        lhs = np.zeros((k, PASS_MOUT[pi]), np.float32)
        r = 0
        for si, (kind, _) in enumerate(spec):
            Ts, Td, v = _slot_tables(T, kind)
            lhs[r:r + 20, 20 * si:20 * si + 20] = Ts
            r += 20
            if Td is not None:
                lhs[r:r + 20, 20 * si:20 * si + 20] = Td
                r += 20
        for si, (kind, _) in enumerate(spec):
            _, _, v = _slot_tables(T, kind)
            lhs[koh + si, 20 * si:20 * si + 20] = v / ARM_SCALE
        w[f'w_pass{pi}'] = lhs.astype(_BF16)

    # S (counts) matmul: rows [cnt_as 20][cnt_ad 20][cnt_ts 20][cnt_td 20][cnt_d 20]
    #                         [sum_a][sum_t][sum_d]; output scaled by 1/24 -> mean
    ws = np.zeros((103, 32), np.float32)
    ws[0:20, 0:20] = T['TAs']
    ws[20:40, 0:20] = T['TAd']
    ws[40:60, 0:20] = T['TTs']
    ws[60:80, 0:20] = T['TTd']
    ws[80:100, 0:20] = T['TD']
    ws[100, 0:20] = T['va'] / SUM_SCALE
    ws[101, 0:20] = T['vt'] / SUM_SCALE
    ws[102, 0:20] = T['vd'] / SUM_SCALE
    w['w_s'] = (ws / 24.0).astype(_BF16)

    # reduce matmuls (block-sum of squares, scaled 1/24). Two row layouts:
    # wr_a for sq tiles with blocks at rows [0:60] and [64:124]
    wra = np.zeros((124, 32), np.float32)
    for r in range(60):
        wra[r, r % 20] = 1.0 / 24.0
    for r in range(64, 124):
        wra[r, (r - 64) % 20] = 1.0 / 24.0
    w['w_r_a'] = wra.astype(_BF16)
    # wr_b for sq tile with blocks at rows [0:120]
    wrb = np.zeros((120, 32), np.float32)
    for r in range(120):
        wrb[r, r % 20] = 1.0 / 24.0
    w['w_r_b'] = wrb.astype(_BF16)

    # p matmul: pv(128, n) -> p(n_chunks_in_group, n);  rows 32g+c map 24*gp[c] to col g
    wgp = np.zeros((128, 4), np.float32)
    for g in range(4):
        wgp[32 * g:32 * g + 20, g] = 24.0 * T['gp']
    w['w_gp'] = wgp.astype(_BF16)

    c2 = np.zeros((128, 1), np.float32)
    for g in range(4):
        c2[32 * g:32 * g + 20, 0] = T['coefA']
    w['c2col'] = c2
    return w


def _pack_core(idx_arrs, arm_arrs):
    """Per-core uint8 pass tensors + counts tensor.

    idx_arrs: dict kind -> (src, dst) or (tgt,) int arrays of shape (MC, 8)
    arm_arrs: dict kind -> (MC, 8) float arrays
    returns dict name -> np.ndarray
    """
    out = {}
    iota = np.arange(N, dtype=np.int32)
    armq = {k: np.clip(np.rint(arm_arrs[k] * ARM_SCALE), 0, 255).astype(np.uint8)
            for k in 'atd'}

    def pad(a):  # (MC, ...) -> (MC_PAD, ...)
        if a.shape[0] == MC_PAD:
            return a
        pw = [(0, MC_PAD - a.shape[0])] + [(0, 0)] * (a.ndim - 1)
        return np.pad(a, pw)

    for pi, spec in enumerate(PASS_SPECS):
        koh, k = PASS_K[pi]
        rows = np.zeros((k, MC_PAD), np.uint8)
        r = 0
        for kind, j in spec:
            srcs = idx_arrs[kind]
            oh = (pad(srcs[0][:, j])[None, :] == iota[:, None]).astype(np.uint8)
            rows[r:r + 20] = oh
            r += 20
            if len(srcs) > 1:
                oh = (pad(srcs[1][:, j])[None, :] == iota[:, None]).astype(np.uint8)
                rows[r:r + 20] = oh
                r += 20
        for si, (kind, j) in enumerate(spec):
            rows[koh + si] = pad(armq[kind][:, j])
        out[f'pass{pi}'] = rows

    cnt = np.zeros((103, MC_PAD), np.uint8)
    pairs = [('a', 0), ('a', 1), ('t', 0), ('t', 1), ('d', 0)]
    for bi, (kind, which) in enumerate(pairs):
        idx = pad(idx_arrs[kind][which])              # (MC_PAD, 8)
        oh = (idx[:, :, None] == iota[None, None, :]).sum(1)  # (MC_PAD, 20) counts
        cnt[20 * bi:20 * bi + 20] = oh.T.astype(np.uint8)
    for bi, kind in enumerate('atd'):
        s = pad(arm_arrs[kind].sum(1))
        cnt[100 + bi] = np.clip(np.rint(s * SUM_SCALE), 0, 255).astype(np.uint8)
    out['counts'] = cnt
    return out


# ------------------------------------------------------------- bass program
def _build_program():
    import concourse.bass as bass
    import concourse.tile as tile
    from concourse import bacc, mybir
    from contextlib import ExitStack

    dt = mybir.dt
    Alu = mybir.AluOpType
    Act = mybir.ActivationFunctionType

    nc = bacc.Bacc('TRN2')
    # inputs
    d_pass = []
    for pi in range(7):
        _, k = PASS_K[pi]
        d_pass.append(nc.declare_dram_parameter(
            f'pass{pi}', [k, MC_PAD], dt.uint8, isOutput=False))
    d_cnt = nc.declare_dram_parameter('counts', [103, MC_PAD], dt.uint8, isOutput=False)
    d_w = []
    for pi in range(7):
        _, k = PASS_K[pi]
        d_w.append(nc.declare_dram_parameter(
            f'w_pass{pi}', [k, PASS_MOUT[pi]], dt.bfloat16, isOutput=False))
    d_ws = nc.declare_dram_parameter('w_s', [103, 32], dt.bfloat16, isOutput=False)
    d_wra = nc.declare_dram_parameter('w_r_a', [124, 32], dt.bfloat16, isOutput=False)
    d_wrb = nc.declare_dram_parameter('w_r_b', [120, 32], dt.bfloat16, isOutput=False)
    d_wgp = nc.declare_dram_parameter('w_gp', [128, 4], dt.bfloat16, isOutput=False)
    d_c2 = nc.declare_dram_parameter('c2col', [128, 1], dt.float32, isOutput=False)
    d_cpeps = nc.declare_dram_parameter('cp_eps', [128, 3], dt.float32, isOutput=False)
    d_out = nc.declare_dram_parameter('p2d', [NCHUNK, CHUNK], dt.float32, isOutput=True)

    with tile.TileContext(nc) as tc, ExitStack() as ctx:
        wpool = ctx.enter_context(tc.tile_pool(name='weights', bufs=1))
        rhs_pool = ctx.enter_context(tc.tile_pool(name='rhs', bufs=2))
        sq_pool = ctx.enter_context(tc.tile_pool(name='sq', bufs=2))
        pst_pool = ctx.enter_context(tc.tile_pool(name='pstage', bufs=2))
        out_pool = ctx.enter_context(tc.tile_pool(name='outp', bufs=2))
        gpsum = ctx.enter_context(tc.tile_pool(name='gpsum', bufs=1, space='PSUM'))
        spsum = ctx.enter_context(tc.tile_pool(name='spsum', bufs=1, space='PSUM'))
        qpsum = ctx.enter_context(tc.tile_pool(name='qpsum', bufs=1, space='PSUM'))
        ppsum = ctx.enter_context(tc.tile_pool(name='ppsum', bufs=1, space='PSUM'))

        # load weights/constants once
        w_sb = []
        for pi in range(7):
            _, k = PASS_K[pi]
            t = wpool.tile([k, PASS_MOUT[pi]], dt.bfloat16, tag=f'w{pi}')
            nc.sync.dma_start(t[:], d_w[pi][:, :])
            w_sb.append(t)
        ws_sb = wpool.tile([103, 32], dt.bfloat16, tag='ws')
        nc.sync.dma_start(ws_sb[:], d_ws[:, :])
        wra_sb = wpool.tile([124, 32], dt.bfloat16, tag='wra')
        nc.sync.dma_start(wra_sb[:], d_wra[:, :])
        wrb_sb = wpool.tile([120, 32], dt.bfloat16, tag='wrb')
        nc.sync.dma_start(wrb_sb[:], d_wrb[:, :])
        wgp_sb = wpool.tile([128, 4], dt.bfloat16, tag='wgp')
        nc.sync.dma_start(wgp_sb[:], d_wgp[:, :])
        c2_sb = wpool.tile([128, 1], dt.float32, tag='c2')
        nc.sync.dma_start(c2_sb[:], d_c2[:, :])
        cpeps_sb = wpool.tile([128, 3], dt.float32, tag='cpeps')
        nc.sync.dma_start(cpeps_sb[:], d_cpeps[:, :])

        half = (NCHUNK // 2) * CHUNK
        rhs_all = []
        for pi in range(7):
            _, k = PASS_K[pi]
            t = wpool.tile([k, MC_PAD], dt.bfloat16, tag=f'rhsall{pi}')
            nc.gpsimd.dma_start(out=t[:, 0:half], in_=d_pass[pi][:, 0:half])
            nc.gpsimd.dma_start(out=t[:, half:MC_PAD], in_=d_pass[pi][:, half:MC_PAD])
            rhs_all.append(t)
        cnt_all = wpool.tile([103, MC_PAD], dt.bfloat16, tag='rhsallcnt')
        nc.gpsimd.dma_start(out=cnt_all[:, 0:half], in_=d_cnt[:, 0:half])
        nc.gpsimd.dma_start(out=cnt_all[:, half:MC_PAD], in_=d_cnt[:, half:MC_PAD])

        for group in GROUPS:
            glen = len(group)
            s_grp = pst_pool.tile([32 * glen, CHUNK], dt.float32, tag='sgrp')
            q_grp = pst_pool.tile([32 * glen, CHUNK], dt.float32, tag='qgrp')
            for gi, c in enumerate(group):
                off = c * CHUNK
                rhs = [rhs_all[pi][:, off:off + CHUNK] for pi in range(7)]
                cnt_t = cnt_all[:, off:off + CHUNK]

                # --- gather matmuls into paired PSUM tiles
                g01 = gpsum.tile([124, CHUNK], dt.float32, tag='g01')
                nc.tensor.matmul(g01[0:60, :], w_sb[0][:], rhs[0], start=True, stop=True)
                nc.tensor.matmul(g01[64:124, :], w_sb[1][:], rhs[1], start=True, stop=True,
                                 tile_position=(0, 64))
                g23 = gpsum.tile([124, CHUNK], dt.float32, tag='g23')
                nc.tensor.matmul(g23[0:60, :], w_sb[2][:], rhs[2], start=True, stop=True)
                nc.tensor.matmul(g23[64:124, :], w_sb[3][:], rhs[3], start=True, stop=True,
                                 tile_position=(0, 64))
                g45 = gpsum.tile([124, CHUNK], dt.float32, tag='g45')
                nc.tensor.matmul(g45[0:60, :], w_sb[4][:], rhs[4], start=True, stop=True)
                nc.tensor.matmul(g45[64:124, :], w_sb[5][:], rhs[5], start=True, stop=True,
                                 tile_position=(0, 64))
                g6 = gpsum.tile([120, CHUNK], dt.float32, tag='g6')
                nc.tensor.matmul(g6[:, :], w_sb[6][:], rhs[6], start=True, stop=True)

                # --- S (mean) matmul (per-chunk psum, copied into SBUF group buf)
                s_ps = spsum.tile([32, CHUNK], dt.float32, tag='sps')
                nc.tensor.matmul(s_ps[:], ws_sb[:], cnt_t, start=True, stop=True)
                nc.vector.tensor_copy(s_grp[32 * gi:32 * gi + 32, :], s_ps[:])

                # --- squares (PSUM -> SBUF bf16), alternating engines
                sq0 = sq_pool.tile([124, CHUNK], dt.bfloat16, tag='sq0')
                nc.scalar.square(sq0[:], g01[:])
                sq1 = sq_pool.tile([124, CHUNK], dt.bfloat16, tag='sq1')
                nc.scalar.square(sq1[:], g23[:])
                sq2 = sq_pool.tile([124, CHUNK], dt.bfloat16, tag='sq2')
                nc.scalar.square(sq2[:], g45[:])
                sq3 = sq_pool.tile([120, CHUNK], dt.bfloat16, tag='sq3')
                nc.scalar.square(sq3[:], g6[:, :])

                # --- Q reduce matmuls (accumulate in per-chunk psum, copy to group buf)
                q_ps = qpsum.tile([32, CHUNK], dt.float32, tag='qps')
                nc.tensor.matmul(q_ps[:], wra_sb[:], sq0[:], start=True, stop=False)
                nc.tensor.matmul(q_ps[:], wra_sb[:], sq1[:], start=False, stop=False)
                nc.tensor.matmul(q_ps[:], wra_sb[:], sq2[:], start=False, stop=False)
                nc.tensor.matmul(q_ps[:], wrb_sb[:], sq3[:], start=False, stop=True)
                nc.vector.tensor_copy(q_grp[32 * gi:32 * gi + 32, :], q_ps[:])

            # --- normalization + p for the whole group (128 partitions)
            rows = 32 * glen
            ss = pst_pool.tile([rows, CHUNK], dt.float32, tag='ss')
            nc.scalar.square(ss[:], s_grp[:])                      # mean^2
            sc = pst_pool.tile([rows, CHUNK], dt.float32, tag='sc')
            nc.vector.tensor_scalar(sc[:], ss[:], c2_sb[0:rows, :], None, Alu.mult)
            vs = pst_pool.tile([rows, CHUNK], dt.float32, tag='vs')
            nc.vector.tensor_tensor(vs[:], q_grp[:], sc[:], Alu.subtract)   # var
            rq = pst_pool.tile([rows, CHUNK], dt.float32, tag='rq')
            nc.scalar.activation(rq[:], vs[:], Act.Abs_reciprocal_sqrt,
                                 bias=cpeps_sb[0:rows, 1:2], scale=1.0)
            pv = pst_pool.tile([rows, CHUNK], dt.bfloat16, tag='pv')
            nc.vector.tensor_tensor(pv[:], s_grp[:], rq[:], Alu.mult)
            p_ps = ppsum.tile([glen, CHUNK], dt.float32, tag='pps')
            nc.tensor.matmul(p_ps[:], wgp_sb[0:rows, 0:glen], pv[:], start=True, stop=True)
            p_sb = out_pool.tile([glen, CHUNK], dt.float32, tag='psb')
            nc.scalar.activation(p_sb[:], p_ps[:], Act.Identity,
                                 bias=cpeps_sb[0:glen, 0:1], scale=1.0)
            nc.sync.dma_start(d_out[group[0]:group[0] + glen, :], p_sb[:])

    nc.finalize()
    return nc


# ------------------------------------------------------------------- runner
_CACHE = {}


def _get_program():
    if 'nc' not in _CACHE:
        _CACHE['nc'] = _build_program()
    return _CACHE['nc']


def kernel(**inputs):
    V, logp, _ = _run(inputs)
    return V, logp


def _run(inputs, trace=False, trace_kwargs=None):
    from concourse.bass_utils import run_bass_kernel_spmd

    f = {k: np.asarray(v, np.float32) if np.asarray(v).dtype != np.int32
         else np.asarray(v) for k, v in inputs.items()}
    V, T = _graph_stage(f)
    w = _build_weights(T)
    w['cp_eps'] = np.stack([np.full(128, T['Cp'], np.float32), np.full(128, EPS, np.float32), np.zeros(128, np.float32)], 1)

    in_maps = []
    for core in range(N_CORES):
        sl = slice(core * MC, (core + 1) * MC)
        idx_arrs = {'a': (f['a_src'][sl], f['a_dst'][sl]),
                    't': (f['t_src'][sl], f['t_dst'][sl]),
                    'd': (f['d_tgt'][sl],)}
        arm_arrs = {'a': f['a_armies'][sl], 't': f['t_armies'][sl],
                    'd': f['d_armies'][sl]}
        m = _pack_core(idx_arrs, arm_arrs)
        m.update(w)
        in_maps.append(m)

    nc = _get_program()
    res = run_bass_kernel_spmd(nc, in_maps, list(range(N_CORES)),
                               trace=trace, **(trace_kwargs or {}))
    p = np.concatenate([r['p2d'].reshape(-1)[:MC] for r in res.results])

    mx = p.max()
    logp = (p - mx - np.log(np.exp(p - mx).sum())).astype(np.float32)
    return np.float32(V), logp, res
